# revision 1
# baseline (speedup 1.0000x reference)
"""PosGCN Trainium2 kernel: out = x + relu(segment_sum((x@W)[edge_src], edge_dst) + b).

Distribution: 1D node partition across 8 NeuronCores. Core c owns dst nodes
[c*12500, (c+1)*12500) and the edges incident to them (partitioned by dst).
W/b replicated; x replicated into every core's HBM so cross-partition source
rows are gathered locally (no collectives).

Key algebraic rewrite: aggregation is linear, so
    segment_sum((x@W)[src]) == segment_sum(x[src]) @ W
We aggregate raw x features per dst first (the memory-bound part), then apply
the dense transform to the 12500 aggregated rows per core.

Gather: the production InstDMAGatherAnt (nc.gpsimd.dma_gather) fetches up to
thousands of 512B rows per instruction. Its indices are int16, so x rows are
addressed quadrant-relative (4 quadrants of 25000 rows); each core's edges are
sorted by (dst-chunk, src-quadrant) and each (chunk, quadrant) run is padded
to a multiple of 128 edges (pad: src=quadrant row 0, dst sentinel -> one-hot
row of zeros -> contributes nothing).

Precision: x is split into bf16 hi + bf16 lo (x ~= hi + lo, ~16 mantissa bits)
packed per row as [hi(128) | lo(128)] so a gathered row stays one 512B unit.
The one-hot segment-sum matmuls run in bf16 (exact 0/1 one-hot) accumulating
in f32 PSUM -> ~1e-5 relative error, at full PE rate.
"""

import sys
from contextlib import ExitStack

import numpy as np

sys.path.insert(0, "/opt/trn_rl_repo")

import ml_dtypes

import concourse.bass as bass
import concourse.tile as tile
from concourse import bacc, mybir
from concourse.bass_utils import run_bass_kernel_spmd

P = 128
N_NODES = 100000
N_EDGES = 1600000
D = 128
N_CORES = 8
QN = 25000  # quadrant size (int16 gather indices must stay < 32768)
NQ = 4

f32 = mybir.dt.float32
bf16 = mybir.dt.bfloat16
i16 = mybir.dt.int16

# test.py can read results metadata from here after a run
last_results = None


def _cdiv(a, b):
    return (a + b - 1) // b


def _build_tables(edge_src, edge_dst, n_nodes, n_cores, qn):
    """Sort each core's edges by (dst-chunk, src-quadrant); build gather runs.

    Returns (runs, tile_base, T, per_core) where:
      runs: list of (chunk k, quadrant q, tile_start ts, n_tiles nt) — one
            dma_gather instruction each; identical across cores (SPMD).
      tile_base[k]: first global tile of chunk k.
      per_core[c] = (idx16_tbl [128, T*8] int16, dstr_tbl [128, T] f32).
    Edge-tile t (128 edges) occupies flat slots [t*128, (t+1)*128); lane p of
    tile t is flat slot t*128+p (matches dma_gather's dst[i%128, i//128]).
    """
    NS = n_nodes // n_cores
    CH = _cdiv(NS, P)
    nq = _cdiv(n_nodes, qn)

    order = np.argsort(edge_dst, kind="stable")
    ss = np.asarray(edge_src)[order].astype(np.int64)
    ds = np.asarray(edge_dst)[order].astype(np.int64)
    bounds = np.searchsorted(ds, np.arange(n_cores + 1) * NS)

    counts = np.zeros((n_cores, CH, nq), np.int64)
    segs = []
    for c in range(n_cores):
        b0, b1 = bounds[c], bounds[c + 1]
        src_c = ss[b0:b1]
        ldst = ds[b0:b1] - c * NS
        quad = src_c // qn
        o2 = np.lexsort((quad, ldst // P))
        src_c, ldst, quad = src_c[o2], ldst[o2], quad[o2]
        np.add.at(counts[c], (ldst // P, quad), 1)
        segs.append((src_c, ldst))

    tiles_kq = _cdiv(counts.max(axis=0), P)  # [CH, nq]
    for k in range(CH):
        if tiles_kq[k].sum() == 0:
            tiles_kq[k][0] = 1  # keep >=1 tile so the chunk's PSUM is written

    runs = []
    tile_base = np.zeros(CH + 1, np.int64)
    t = 0
    for k in range(CH):
        tile_base[k] = t
        for q in range(nq):
            nt = int(tiles_kq[k][q])
            if nt:
                runs.append((k, q, t, nt))
                t += nt
    tile_base[CH] = t
    T = t

    per_core = []
    for c in range(n_cores):
        src_c, ldst = segs[c]
        # per-(k,q) offsets into the sorted edge arrays
        offs = np.zeros((CH, nq + 1), np.int64)
        offs[:, 1:] = np.cumsum(counts[c], axis=1)
        row_off = np.concatenate([[0], np.cumsum(counts[c].sum(axis=1))])
        flat_src = np.zeros(T * P, np.int16)
        flat_dstr = np.full(T * P, float(P), np.float32)
        for k, q, ts, nt in runs:
            n = int(counts[c][k][q])
            if n == 0:
                continue
            s = int(row_off[k] + offs[k][q])
            slot = ts * P
            flat_src[slot : slot + n] = (src_c[s : s + n] - q * qn).astype(np.int16)
            flat_dstr[slot : slot + n] = (ldst[s : s + n] % P).astype(np.float32)
        # dma_gather wrapped-index layout: idxs[p, s] = flat[s*16 + p], p<16,
        # replicated over the other 112 partitions
        idx16 = np.tile(flat_src.reshape(T * 8, 16).T, (8, 1))
        per_core.append(
            (
                np.ascontiguousarray(idx16),
                np.ascontiguousarray(flat_dstr.reshape(T, P).T),
            )
        )
    return runs, tile_base, T, per_core


def _build_program(n_nodes, NS, runs, tile_base, T, qn, reps=1, skip=()):
    """Emit the SPMD Bass program for one core (identical across cores).

    reps > 1 replicates the whole body (for slope-based HW timing; dispatch
    overhead cancels between two rep counts).
    """
    CH = len(tile_base) - 1
    last_w = NS - (CH - 1) * P

    nc = bacc.Bacc(
        "TRN2",
        target_bir_lowering=False,
        debug=False,
        num_devices=N_CORES,
        num_swdge_queues=4,
    )

    xp = nc.dram_tensor("xp", [n_nodes, 2 * D], bf16, kind="ExternalInput")
    idx16 = nc.dram_tensor("idx16", [P, T * 8], i16, kind="ExternalInput")
    dstr = nc.dram_tensor("dstr", [P, T], f32, kind="ExternalInput")
    xrt = nc.dram_tensor("xrt", [P, NS], f32, kind="ExternalInput")
    w = nc.dram_tensor("w", [P, D], f32, kind="ExternalInput")
    bcol = nc.dram_tensor("b", [P, 1], f32, kind="ExternalInput")
    iota = nc.dram_tensor("iota", [P, P], f32, kind="ExternalInput")
    outT = nc.dram_tensor("outT", [P, NS], f32, kind="ExternalOutput")

    # tile t -> (its gather run, index within run)
    tile_run = {}
    for ri, (k, q, ts, nt) in enumerate(runs):
        for j in range(nt):
            tile_run[ts + j] = (ri, j)

    with tile.TileContext(nc) as tc, ExitStack() as ctx:
        const = ctx.enter_context(tc.tile_pool(name="const", bufs=1))
        tbl = ctx.enter_context(tc.tile_pool(name="tbl", bufs=1))
        gpool = ctx.enter_context(tc.tile_pool(name="gather", bufs=4))
        opool = ctx.enter_context(tc.tile_pool(name="onehot", bufs=6))
        apsum = ctx.enter_context(tc.tile_pool(name="apsum", bufs=2, space="PSUM"))
        opsum = ctx.enter_context(tc.tile_pool(name="opsum", bufs=2, space="PSUM"))
        asb = ctx.enter_context(tc.tile_pool(name="asb", bufs=2))
        osb = ctx.enter_context(tc.tile_pool(name="osb", bufs=3))
        xrp = ctx.enter_context(tc.tile_pool(name="xrp", bufs=3))

        def emit_body():
            w_sb = const.tile([P, D], f32)
            nc.sync.dma_start(out=w_sb[:], in_=w[:])
            b_sb = const.tile([P, 1], f32)
            nc.sync.dma_start(out=b_sb[:], in_=bcol[:])
            io_sb = const.tile([P, P], f32)
            nc.sync.dma_start(out=io_sb[:], in_=iota[:])
            idx_sb = tbl.tile([P, T * 8], i16)
            nc.sync.dma_start(out=idx_sb[:], in_=idx16[:])
            dstr_sb = tbl.tile([P, T], f32)
            nc.sync.dma_start(out=dstr_sb[:], in_=dstr[:])

            gbufs = {}
            if "onehot" in skip:
                ohc = const.tile([P, P], bf16, tag="ohc")
                nc.vector.memset(ohc[:], 0.0)
            if "gather" in skip:
                dummy = const.tile([P, 1, 2 * D], bf16, tag="dummy")
                nc.vector.memset(dummy[:], 0.0)

            def gather_run(ri):
                k, q, ts, nt = runs[ri]
                if "gather" in skip:
                    gbufs[ri] = dummy
                    return
                gdim = D if "half" in skip else 2 * D
                gb = gpool.tile([P, nt, gdim], bf16, tag="gb")
                halves = 2 if "split2" in skip else 1
                qnum = 0 if "queue0" in skip else ri % 4
                bounds_t = [0, nt] if halves == 1 else [0, (nt + 1) // 2, nt]
                for h in range(halves):
                    a, bnd = bounds_t[h], bounds_t[h + 1]
                    if bnd == a:
                        continue
                    if "half" in skip:
                        nc.gpsimd.dma_gather(
                            gb[:, a:bnd, :],
                            xp[q * qn : (q + 1) * qn, 0:D],
                            idx_sb[:, (ts + a) * 8 : (ts + bnd) * 8],
                            (bnd - a) * P,
                            (bnd - a) * P,
                            D,
                            elem_step=2 * D,
                            queue_num=qnum,
                        )
                    else:
                        nc.gpsimd.dma_gather(
                            gb[:, a:bnd, :],
                            xp[q * qn : (q + 1) * qn, :],
                            idx_sb[:, (ts + a) * 8 : (ts + bnd) * 8],
                            (bnd - a) * P,
                            (bnd - a) * P,
                            2 * D,
                            elem_step=2 * D,
                            queue_num=qnum,
                        )
                gbufs[ri] = gb

            for k in range(CH):
                t0, t1 = int(tile_base[k]), int(tile_base[k + 1])
                nt_k = t1 - t0
                psum = apsum.tile([P, P], f32)
                if "mm" in skip:
                    nc.vector.memset(psum[:], 0.0)
                for j in range(nt_k):
                    t = t0 + j
                    ri, jr = tile_run[t]
                    if ri not in gbufs:
                        gather_run(ri)
                    gb = gbufs[ri]
                    oh = ohc if "onehot" in skip else opool.tile([P, P], bf16)
                    if "onehot" not in skip:
                        nc.vector.tensor_tensor(
                            out=oh[:],
                            in0=dstr_sb[:, t : t + 1].to_broadcast([P, P]),
                            in1=io_sb[:],
                            op=mybir.AluOpType.is_equal,
                        )
                    if "mm" not in skip:
                        jg = 0 if "gather" in skip else jr
                        hi = gb[:, jg, 0:D]
                        lo = hi if "half" in skip else gb[:, jg, D : 2 * D]
                        nc.tensor.matmul(
                            out=psum[:], lhsT=hi, rhs=oh[:], start=(j == 0), stop=False
                        )
                        nc.tensor.matmul(
                            out=psum[:],
                            lhsT=lo,
                            rhs=oh[:],
                            start=False,
                            stop=(j == nt_k - 1),
                        )

                # aggT[f, d] (PSUM) -> out2T[g, d] = sum_f W[f,g] * aggT[f,d]
                aggT = asb.tile([P, P], f32)
                nc.vector.tensor_copy(out=aggT[:], in_=psum[:])
                po = opsum.tile([P, P], f32)
                nc.tensor.matmul(out=po[:], lhsT=w_sb[:], rhs=aggT[:], start=True, stop=True)
                ot = osb.tile([P, P], f32)
                nc.scalar.activation(
                    out=ot[:],
                    in_=po[:],
                    func=mybir.ActivationFunctionType.Relu,
                    bias=b_sb[:],
                )
                wk = P if k < CH - 1 else last_w
                xr = xrp.tile([P, P], f32)
                nc.sync.dma_start(out=xr[:, :wk], in_=xrt[:, k * P : k * P + wk])
                nc.vector.tensor_add(out=ot[:, :wk], in0=ot[:, :wk], in1=xr[:, :wk])
                nc.sync.dma_start(out=outT[:, k * P : k * P + wk], in_=ot[:, :wk])

        for _rep in range(reps):
            emit_body()

    nc.compile()
    return nc


def _make_in_maps(x, edge_src, edge_dst, W, b, n_nodes, n_cores, qn):
    NS = n_nodes // n_cores
    xf = np.ascontiguousarray(np.asarray(x, dtype=np.float32))
    x_hi = xf.astype(ml_dtypes.bfloat16)
    x_lo = (xf - x_hi.astype(np.float32)).astype(ml_dtypes.bfloat16)
    xp = np.ascontiguousarray(np.concatenate([x_hi, x_lo], axis=1))

    runs, tile_base, T, per_core = _build_tables(
        np.asarray(edge_src), np.asarray(edge_dst), n_nodes, n_cores, qn
    )

    w_np = np.ascontiguousarray(np.asarray(W, dtype=np.float32))
    b_np = np.ascontiguousarray(np.asarray(b, dtype=np.float32).reshape(P, 1))
    iota_np = np.ascontiguousarray(
        np.broadcast_to(np.arange(P, dtype=np.float32), (P, P))
    )

    in_maps = []
    for c in range(n_cores):
        idx16_tbl, dstr_tbl = per_core[c]
        in_maps.append(
            {
                "xp": xp,
                "idx16": idx16_tbl,
                "dstr": dstr_tbl,
                "xrt": np.ascontiguousarray(xf[c * NS : (c + 1) * NS].T),
                "w": w_np,
                "b": b_np,
                "iota": iota_np,
            }
        )
    return in_maps, runs, tile_base, T


def prepare(x, edge_src, edge_dst, W, b):
    """Build (nc, in_maps) for the 8-core SPMD run."""
    n_nodes = int(np.asarray(x).shape[0])
    NS = n_nodes // N_CORES
    qn = min(QN, n_nodes)
    in_maps, runs, tile_base, T = _make_in_maps(
        x, edge_src, edge_dst, W, b, n_nodes, N_CORES, qn
    )
    nc = _build_program(n_nodes, NS, runs, tile_base, T, qn)
    return nc, in_maps


def kernel(x, edge_src, edge_dst, W, b):
    global last_results
    n_nodes = int(np.asarray(x).shape[0])
    NS = n_nodes // N_CORES

    nc, in_maps = prepare(x, edge_src, edge_dst, W, b)
    res = run_bass_kernel_spmd(nc, in_maps, core_ids=list(range(N_CORES)))
    last_results = res

    out = np.empty((n_nodes, D), np.float32)
    for c in range(N_CORES):
        out[c * NS : (c + 1) * NS] = res.results[c]["outT"].T
    return out



# revision 2
# speedup vs baseline: 1.0034x; 1.0034x over previous
"""PosGCN Trainium2 kernel v2: out = x + relu(segment_sum((x@W)[edge_src], edge_dst) + b).

Distribution: 1D node partition across 8 NeuronCores. Core c owns dst nodes
[c*12500, (c+1)*12500) and the edges incident to them (partitioned by dst).
W/b replicated; x replicated into every core's HBM so cross-partition source
rows are gathered locally (no collectives).

Aggregation is linear, so segment_sum((x@W)[src]) == segment_sum(x[src]) @ W.
We aggregate raw x features per dst first, then apply the dense transform to
the 12500 aggregated rows per core.

Key structure (arrived at via HW ablation - the bottleneck is the serial
per-index SWDGE descriptor generation on the GPSIMD Q7 cores, ~2.5ns/idx,
insensitive to transfer bytes and queue count):
- hi-only bf16 gather (256B rows): tolerance is 2e-2, bf16 aggregation error
  is ~1.5e-3. Halves matmul count and HBM traffic vs an f32 or hi+lo gather.
- grouped gathers (CG chunks per group x 4 quadrants), each split into
  <=1024-index dma_gather instructions (the HW limit is between 1024 and
  1536 indices per instruction; larger gathers hang the device).
- 16 gather buffers: deep pipeline so gathers run ahead of consumption
  (measured ~12% faster than 8).
- batched one-hot builds: one DVE is_equal builds OG=16 tiles at once via
  stride-0 broadcast 3D APs, bf16 in/out.
- PSUM->SBUF aggregate copy on the Activation engine (Copy activation),
  keeping the DVE for one-hots and residual adds.

Gather: nc.gpsimd.dma_gather with int16 indices, so x rows are addressed
quadrant-relative (4 quadrants of 25000 rows); each core's edges are sorted
by (chunk-group, src-quadrant, chunk) and each (chunk, quadrant) run is
padded to a multiple of 128 edges (pad: src=quadrant row 0, dst sentinel ->
one-hot row of zeros -> contributes nothing).
"""

import sys
from contextlib import ExitStack

import numpy as np

sys.path.insert(0, "/opt/trn_rl_repo")

import ml_dtypes

import concourse.bass as bass
import concourse.tile as tile
from concourse import bacc, mybir
from concourse.bass_utils import run_bass_kernel_spmd

P = 128
N_NODES = 100000
N_EDGES = 1600000
D = 128
N_CORES = 8
QN = 25000  # quadrant size (int16 gather indices must stay < 32768)
NQ = 4
CG = 6  # chunks per gather group (keeps num_idxs/16 well under the SWDGE ring)
OG = 16  # tiles per batched one-hot build
MAXT = 8  # max tiles (x128 idxs) per dma_gather instruction

f32 = mybir.dt.float32
bf16 = mybir.dt.bfloat16
i16 = mybir.dt.int16

last_results = None


def _cdiv(a, b):
    return (a + b - 1) // b


def _build_tables(edge_src, edge_dst, n_nodes, n_cores, qn):
    """Sort each core's edges by (chunk-group, src-quadrant, chunk); build
    grouped gather runs.

    Tile layout (identical across cores, SPMD): for each group g of CG chunks,
    for each quadrant q, the (k, q) cells of the group's chunks are laid out
    contiguously, each padded to tiles_kq[k][q] tiles of 128 edges. One
    dma_gather instruction covers each (g, q) region.

    Consumption order (matmul order): chunk-major - for each chunk k, its
    tiles across the 4 quadrant regions in (q, tile) order. The dstr table is
    stored in consumption order so batched one-hot builds read contiguous
    columns; cons2slot maps consumption position -> gather tile index.

    Returns (runs, chunk_tiles, T, per_core):
      runs: list of (g, q, ts, nt) - one dma_gather each.
      chunk_tiles: list over chunks of lists of gather-tile indices.
      T: total gather tiles.
      per_core[c] = (idx16_tbl [128, T*8] int16, dstr_tbl [128, T] f32)
        with dstr in consumption order.
    """
    NS = n_nodes // n_cores
    CH = _cdiv(NS, P)
    nq = _cdiv(n_nodes, qn)
    NGRP = _cdiv(CH, CG)

    order = np.argsort(edge_dst, kind="stable")
    ss = np.asarray(edge_src)[order].astype(np.int64)
    ds = np.asarray(edge_dst)[order].astype(np.int64)
    bounds = np.searchsorted(ds, np.arange(n_cores + 1) * NS)

    counts = np.zeros((n_cores, CH, nq), np.int64)
    segs = []
    for c in range(n_cores):
        b0, b1 = bounds[c], bounds[c + 1]
        src_c = ss[b0:b1]
        ldst = ds[b0:b1] - c * NS
        chunk = ldst // P
        quad = src_c // qn
        o2 = np.lexsort((quad, chunk))  # chunk-major, quadrant within chunk
        src_c, ldst, chunk, quad = src_c[o2], ldst[o2], chunk[o2], quad[o2]
        np.add.at(counts[c], (chunk, quad), 1)
        segs.append((src_c, ldst))

    tiles_kq = _cdiv(counts.max(axis=0), P)  # [CH, nq]
    for k in range(CH):
        if tiles_kq[k].sum() == 0:
            tiles_kq[k][0] = 1  # keep >=1 tile so the chunk's PSUM is written

    # Gather-tile layout + runs
    runs = []
    cell_tile = np.full((CH, nq), -1, np.int64)  # first tile of cell (k, q)
    t = 0
    for g in range(NGRP):
        ks = range(g * CG, min((g + 1) * CG, CH))
        for q in range(nq):
            ts = t
            for k in ks:
                nt = int(tiles_kq[k][q])
                if nt:
                    cell_tile[k][q] = t
                    t += nt
            if t > ts:
                runs.append((g, q, ts, t - ts))
    T = t

    # Consumption order: chunk-major over its cells' tiles
    chunk_tiles = []
    cons2slot = np.zeros(T, np.int64)  # consumption position -> gather tile
    p = 0
    for k in range(CH):
        tl = []
        for q in range(nq):
            for j in range(int(tiles_kq[k][q])):
                tl.append(int(cell_tile[k][q]) + j)
        chunk_tiles.append(tl)
        for gt in tl:
            cons2slot[p] = gt
            p += 1
    assert p == T

    per_core = []
    for c in range(n_cores):
        src_c, ldst = segs[c]
        offs = np.zeros((CH, nq + 1), np.int64)
        offs[:, 1:] = np.cumsum(counts[c], axis=1)
        row_off = np.concatenate([[0], np.cumsum(counts[c].sum(axis=1))])
        flat_src = np.zeros(T * P, np.int16)
        flat_dstr = np.full(T * P, float(P), np.float32)
        for k in range(CH):
            for q in range(nq):
                n = int(counts[c][k][q])
                if n == 0:
                    continue
                s = int(row_off[k] + offs[k][q])
                slot = int(cell_tile[k][q]) * P
                flat_src[slot : slot + n] = (src_c[s : s + n] - q * qn).astype(
                    np.int16
                )
                flat_dstr[slot : slot + n] = (ldst[s : s + n] % P).astype(np.float32)
        # dma_gather wrapped-index layout: idxs[p, s] = flat[s*16 + p], p<16,
        # replicated over the other 112 partitions
        idx16 = np.tile(flat_src.reshape(T * 8, 16).T, (8, 1))
        # dstr in consumption order
        dstr_cons = flat_dstr.reshape(T, P)[cons2slot]
        per_core.append(
            (
                np.ascontiguousarray(idx16),
                np.ascontiguousarray(dstr_cons.T.astype(ml_dtypes.bfloat16)),
            )
        )
    return runs, chunk_tiles, T, per_core


def _build_program(n_nodes, NS, runs, chunk_tiles, T, qn, skip=()):
    """Emit the SPMD Bass program for one core (identical across cores)."""
    CH = len(chunk_tiles)
    last_w = NS - (CH - 1) * P
    nq = _cdiv(n_nodes, qn)

    nc = bacc.Bacc(
        "TRN2",
        target_bir_lowering=False,
        debug=False,
        num_devices=N_CORES,
        num_swdge_queues=4,
    )

    gd = 2 * D if "elem512" in skip else D
    xq = nc.dram_tensor("xq", [n_nodes, gd], bf16, kind="ExternalInput")
    idx16 = nc.dram_tensor("idx16", [P, T * 8], i16, kind="ExternalInput")
    dstr = nc.dram_tensor("dstr", [P, T], bf16, kind="ExternalInput")
    xrt = nc.dram_tensor("xrt", [P, NS], f32, kind="ExternalInput")
    w = nc.dram_tensor("w", [P, D], f32, kind="ExternalInput")
    bcol = nc.dram_tensor("b", [P, 1], f32, kind="ExternalInput")
    iota = nc.dram_tensor("iota", [P, P], bf16, kind="ExternalInput")
    outT = nc.dram_tensor("outT", [P, NS], f32, kind="ExternalOutput")

    # gather tile -> (run index, offset within run)
    tile_run = {}
    for ri, (g, q, ts, nt) in enumerate(runs):
        for j in range(nt):
            tile_run[ts + j] = (ri, j)
    # chunk -> first run index of its group's runs (for prefetch ordering)
    nruns = len(runs)

    with tile.TileContext(nc) as tc, ExitStack() as ctx:
        const = ctx.enter_context(tc.tile_pool(name="const", bufs=1))
        tbl = ctx.enter_context(tc.tile_pool(name="tbl", bufs=1))
        gbufs_n = 16
        if "bufs4" in skip:
            gbufs_n = 4
        if "bufs8" in skip:
            gbufs_n = 8
        gpool = ctx.enter_context(tc.tile_pool(name="gather", bufs=gbufs_n))
        opool = ctx.enter_context(tc.tile_pool(name="onehot", bufs=4))
        apsum = ctx.enter_context(tc.tile_pool(name="apsum", bufs=2, space="PSUM"))
        opsum = ctx.enter_context(tc.tile_pool(name="opsum", bufs=2, space="PSUM"))
        asb = ctx.enter_context(tc.tile_pool(name="asb", bufs=2))
        osb = ctx.enter_context(tc.tile_pool(name="osb", bufs=3))
        xrp = ctx.enter_context(tc.tile_pool(name="xrp", bufs=3))

        w_sb = const.tile([P, D], f32)
        nc.sync.dma_start(out=w_sb[:], in_=w[:])
        b_sb = const.tile([P, 1], f32)
        nc.sync.dma_start(out=b_sb[:], in_=bcol[:])
        io_sb = const.tile([P, P], bf16)
        nc.sync.dma_start(out=io_sb[:], in_=iota[:])
        idx_sb = tbl.tile([P, T * 8], i16)
        nc.sync.dma_start(out=idx_sb[:], in_=idx16[:])
        dstr_sb = tbl.tile([P, T], bf16)
        nc.sync.dma_start(out=dstr_sb[:], in_=dstr[:])

        gbufs = {}
        if "gather" in skip:
            dummy = const.tile([P, 1, D], bf16, tag="dummy")
            nc.vector.memset(dummy[:], 0.0)
        if "onehot" in skip:
            ohc = const.tile([P, P], bf16, tag="ohc")
            nc.vector.memset(ohc[:], 0.0)

        def gather_run(ri):
            g, q, ts, nt = runs[ri]
            if "gather" in skip:
                gbufs[ri] = dummy
                return
            gb = gpool.tile([P, nt, gd], bf16, tag="gb")
            nq_queues = 2 if "queue2" in skip else 4
            for a in range(0, nt, MAXT):
                bnd = min(a + MAXT, nt)
                nc.gpsimd.dma_gather(
                    gb[:, a:bnd, :],
                    xq[q * qn : (q + 1) * qn, :],
                    idx_sb[:, (ts + a) * 8 : (ts + bnd) * 8],
                    (bnd - a) * P,
                    (bnd - a) * P,
                    gd,
                    elem_step=gd,
                    queue_num=ri % nq_queues,
                )
            gbufs[ri] = gb

        obufs = {}

        def build_onehot(ob):
            """Batched one-hot build for consumption positions [ob*OG, ...)."""
            p0 = ob * OG
            kt = min(OG, T - p0)
            oh = opool.tile([P, OG, P], bf16, tag="oh")
            if "og1" in skip:
                for j in range(kt):
                    nc.vector.tensor_tensor(
                        out=oh[:, j, :],
                        in0=dstr_sb[:, p0 + j : p0 + j + 1].to_broadcast([P, P]),
                        in1=io_sb[:],
                        op=mybir.AluOpType.is_equal,
                    )
            else:
                nc.vector.tensor_tensor(
                    out=oh[:, 0:kt, :],
                    in0=dstr_sb[:, p0 : p0 + kt].unsqueeze(2).broadcast_to([P, kt, P]),
                    in1=io_sb[:].unsqueeze(1).broadcast_to([P, kt, P]),
                    op=mybir.AluOpType.is_equal,
                )
            obufs[ob] = oh

        cons_pos = 0
        for k in range(CH):
            tl = chunk_tiles[k]
            psum = apsum.tile([P, P], f32)
            if "mm" in skip:
                nc.vector.memset(psum[:], 0.0)
            for j, t in enumerate(tl):
                ri, jr = tile_run[t]
                if ri not in gbufs:
                    # issue this run and prefetch the next one
                    gather_run(ri)
                    if ri + 1 < nruns and ri + 1 not in gbufs:
                        gather_run(ri + 1)
                gb = gbufs[ri]
                ob = cons_pos // OG
                if "onehot" in skip:
                    oh_ap = ohc[:]
                else:
                    if ob not in obufs:
                        build_onehot(ob)
                    oh_ap = obufs[ob][:, cons_pos - ob * OG, :]
                if "mm" not in skip:
                    jg = 0 if "gather" in skip else jr
                    nc.tensor.matmul(
                        out=psum[:],
                        lhsT=gb[:, jg, 0:D],
                        rhs=oh_ap,
                        start=(j == 0),
                        stop=(j == len(tl) - 1),
                    )
                cons_pos += 1

            # aggT[f, d] (PSUM) -> SBUF on the Act engine, then
            # out2T[g, d] = sum_f W[f,g] * aggT[f,d]
            aggT = asb.tile([P, P], f32)
            nc.scalar.activation(
                out=aggT[:], in_=psum[:], func=mybir.ActivationFunctionType.Copy
            )
            if "notail" in skip:
                continue
            po = opsum.tile([P, P], f32)
            nc.tensor.matmul(out=po[:], lhsT=w_sb[:], rhs=aggT[:], start=True, stop=True)
            ot = osb.tile([P, P], f32)
            nc.scalar.activation(
                out=ot[:],
                in_=po[:],
                func=mybir.ActivationFunctionType.Relu,
                bias=b_sb[:],
            )
            wk = P if k < CH - 1 else last_w
            if "noresid" not in skip:
                xr = xrp.tile([P, P], f32)
                nc.sync.dma_start(out=xr[:, :wk], in_=xrt[:, k * P : k * P + wk])
                nc.vector.tensor_add(out=ot[:, :wk], in0=ot[:, :wk], in1=xr[:, :wk])
            nc.sync.dma_start(out=outT[:, k * P : k * P + wk], in_=ot[:, :wk])

    nc.compile()
    return nc


def _make_in_maps(x, edge_src, edge_dst, W, b, n_nodes, n_cores, qn, skip=()):
    NS = n_nodes // n_cores
    xf = np.ascontiguousarray(np.asarray(x, dtype=np.float32))
    x_hi = xf.astype(ml_dtypes.bfloat16)
    if "elem512" in skip:
        x_lo = (xf - x_hi.astype(np.float32)).astype(ml_dtypes.bfloat16)
        xq = np.ascontiguousarray(np.concatenate([x_hi, x_lo], axis=1))
    else:
        xq = np.ascontiguousarray(x_hi)

    runs, chunk_tiles, T, per_core = _build_tables(
        np.asarray(edge_src), np.asarray(edge_dst), n_nodes, n_cores, qn
    )

    w_np = np.ascontiguousarray(np.asarray(W, dtype=np.float32))
    b_np = np.ascontiguousarray(np.asarray(b, dtype=np.float32).reshape(P, 1))
    iota_np = np.ascontiguousarray(
        np.broadcast_to(np.arange(P), (P, P)).astype(ml_dtypes.bfloat16)
    )

    in_maps = []
    for c in range(n_cores):
        idx16_tbl, dstr_tbl = per_core[c]
        in_maps.append(
            {
                "xq": xq,
                "idx16": idx16_tbl,
                "dstr": dstr_tbl,
                "xrt": np.ascontiguousarray(xf[c * NS : (c + 1) * NS].T),
                "w": w_np,
                "b": b_np,
                "iota": iota_np,
            }
        )
    return in_maps, runs, chunk_tiles, T


def prepare(x, edge_src, edge_dst, W, b, skip=()):
    """Build (nc, in_maps) for the 8-core SPMD run."""
    n_nodes = int(np.asarray(x).shape[0])
    NS = n_nodes // N_CORES
    qn = min(QN, n_nodes)
    in_maps, runs, chunk_tiles, T = _make_in_maps(
        x, edge_src, edge_dst, W, b, n_nodes, N_CORES, qn, skip=skip
    )
    nc = _build_program(n_nodes, NS, runs, chunk_tiles, T, qn, skip=skip)
    return nc, in_maps


def kernel(x, edge_src, edge_dst, W, b):
    global last_results
    n_nodes = int(np.asarray(x).shape[0])
    NS = n_nodes // N_CORES

    nc, in_maps = prepare(x, edge_src, edge_dst, W, b)
    res = run_bass_kernel_spmd(nc, in_maps, core_ids=list(range(N_CORES)))
    last_results = res

    out = np.empty((n_nodes, D), np.float32)
    for c in range(N_CORES):
        out[c * NS : (c + 1) * NS] = res.results[c]["outT"].T
    return out


# revision 3
# speedup vs baseline: 1.3586x; 1.3539x over previous
"""PosGCN Trainium2 kernel v2: out = x + relu(segment_sum((x@W)[edge_src], edge_dst) + b).

Distribution: 1D node partition across 8 NeuronCores. Core c owns dst nodes
[c*12500, (c+1)*12500) and the edges incident to them (partitioned by dst).
W/b replicated; x replicated into every core's HBM so cross-partition source
rows are gathered locally (no collectives).

Aggregation is linear, so segment_sum((x@W)[src]) == segment_sum(x[src]) @ W.
We aggregate raw x features per dst first, then apply the dense transform to
the 12500 aggregated rows per core.

Key structure (arrived at via HW ablation - the bottleneck is the serial
per-index SWDGE descriptor generation on the GPSIMD Q7 cores, ~2.5ns/idx,
insensitive to transfer bytes and queue count):
- hi-only bf16 gather (256B rows): tolerance is 2e-2, bf16 aggregation error
  is ~1.5e-3. Halves matmul count and HBM traffic vs an f32 or hi+lo gather.
- grouped gathers (CG chunks per group x 4 quadrants), each split into
  <=1024-index dma_gather instructions (the HW limit is between 1024 and
  1536 indices per instruction; larger gathers hang the device).
- 16 gather buffers: deep pipeline so gathers run ahead of consumption
  (measured ~12% faster than 8).
- batched one-hot builds: one DVE is_equal builds OG=16 tiles at once via
  stride-0 broadcast 3D APs, bf16 in/out.
- PSUM->SBUF aggregate copy on the Activation engine (Copy activation),
  keeping the DVE for one-hots and residual adds.

Gather: nc.gpsimd.dma_gather with int16 indices, so x rows are addressed
quadrant-relative (4 quadrants of 25000 rows); each core's edges are sorted
by (chunk-group, src-quadrant, chunk) and each (chunk, quadrant) run is
padded to a multiple of 128 edges (pad: src=quadrant row 0, dst sentinel ->
one-hot row of zeros -> contributes nothing).
"""

import sys
from contextlib import ExitStack

import numpy as np

sys.path.insert(0, "/opt/trn_rl_repo")

import ml_dtypes

import concourse.bass as bass
import concourse.tile as tile
from concourse import bacc, mybir
from concourse.bass_utils import run_bass_kernel_spmd

P = 128
N_NODES = 100000
N_EDGES = 1600000
D = 128
N_CORES = 8
QN = 25000  # quadrant size (int16 gather indices must stay < 32768)
NQ = 4
CG = 6  # chunks per gather group (keeps num_idxs/16 well under the SWDGE ring)
OG = 16  # tiles per batched one-hot build
MAXT = 8  # max tiles (x128 idxs) per dma_gather instruction

f32 = mybir.dt.float32
bf16 = mybir.dt.bfloat16
i16 = mybir.dt.int16

last_results = None


def _cdiv(a, b):
    return (a + b - 1) // b


def _build_tables(edge_src, edge_dst, n_nodes, n_cores, qn):
    """Sort each core's edges by (chunk-group, src-quadrant, chunk); build
    grouped gather runs.

    Tile layout (identical across cores, SPMD): for each group g of CG chunks,
    for each quadrant q, the (k, q) cells of the group's chunks are laid out
    contiguously, each padded to tiles_kq[k][q] tiles of 128 edges. One
    dma_gather instruction covers each (g, q) region.

    Consumption order (matmul order): chunk-major - for each chunk k, its
    tiles across the 4 quadrant regions in (q, tile) order. The dstr table is
    stored in consumption order so batched one-hot builds read contiguous
    columns; cons2slot maps consumption position -> gather tile index.

    Returns (runs, chunk_tiles, T, per_core):
      runs: list of (g, q, ts, nt) - one dma_gather each.
      chunk_tiles: list over chunks of lists of gather-tile indices.
      T: total gather tiles.
      per_core[c] = (idx16_tbl [128, T*8] int16, dstr_tbl [128, T] f32)
        with dstr in consumption order.
    """
    NS = n_nodes // n_cores
    CH = _cdiv(NS, P)
    nq = _cdiv(n_nodes, qn)
    NGRP = _cdiv(CH, CG)

    order = np.argsort(edge_dst, kind="stable")
    ss = np.asarray(edge_src)[order].astype(np.int64)
    ds = np.asarray(edge_dst)[order].astype(np.int64)
    bounds = np.searchsorted(ds, np.arange(n_cores + 1) * NS)

    counts = np.zeros((n_cores, CH, nq), np.int64)
    segs = []
    for c in range(n_cores):
        b0, b1 = bounds[c], bounds[c + 1]
        src_c = ss[b0:b1]
        ldst = ds[b0:b1] - c * NS
        chunk = ldst // P
        quad = src_c // qn
        o2 = np.lexsort((quad, chunk))  # chunk-major, quadrant within chunk
        src_c, ldst, chunk, quad = src_c[o2], ldst[o2], chunk[o2], quad[o2]
        np.add.at(counts[c], (chunk, quad), 1)
        segs.append((src_c, ldst))

    tiles_kq = _cdiv(counts.max(axis=0), P)  # [CH, nq]
    for k in range(CH):
        if tiles_kq[k].sum() == 0:
            tiles_kq[k][0] = 1  # keep >=1 tile so the chunk's PSUM is written

    # Gather-tile layout + runs
    runs = []
    cell_tile = np.full((CH, nq), -1, np.int64)  # first tile of cell (k, q)
    t = 0
    for g in range(NGRP):
        ks = range(g * CG, min((g + 1) * CG, CH))
        for q in range(nq):
            ts = t
            for k in ks:
                nt = int(tiles_kq[k][q])
                if nt:
                    cell_tile[k][q] = t
                    t += nt
            if t > ts:
                runs.append((g, q, ts, t - ts))
    T = t

    # Consumption order: chunk-major over its cells' tiles
    chunk_tiles = []
    cons2slot = np.zeros(T, np.int64)  # consumption position -> gather tile
    p = 0
    for k in range(CH):
        tl = []
        for q in range(nq):
            for j in range(int(tiles_kq[k][q])):
                tl.append(int(cell_tile[k][q]) + j)
        chunk_tiles.append(tl)
        for gt in tl:
            cons2slot[p] = gt
            p += 1
    assert p == T

    per_core = []
    for c in range(n_cores):
        src_c, ldst = segs[c]
        offs = np.zeros((CH, nq + 1), np.int64)
        offs[:, 1:] = np.cumsum(counts[c], axis=1)
        row_off = np.concatenate([[0], np.cumsum(counts[c].sum(axis=1))])
        flat_src = np.zeros(T * P, np.int16)
        flat_dstr = np.full(T * P, float(P), np.float32)
        for k in range(CH):
            for q in range(nq):
                n = int(counts[c][k][q])
                if n == 0:
                    continue
                s = int(row_off[k] + offs[k][q])
                slot = int(cell_tile[k][q]) * P
                flat_src[slot : slot + n] = (src_c[s : s + n] - q * qn).astype(
                    np.int16
                )
                flat_dstr[slot : slot + n] = (ldst[s : s + n] % P).astype(np.float32)
        # dma_gather wrapped-index layout: idxs[p, s] = flat[s*16 + p], p<16,
        # replicated over the other 112 partitions
        idx16 = np.tile(flat_src.reshape(T * 8, 16).T, (8, 1))
        # dstr in consumption order
        dstr_cons = flat_dstr.reshape(T, P)[cons2slot]
        per_core.append(
            (
                np.ascontiguousarray(idx16),
                np.ascontiguousarray(dstr_cons.T.astype(ml_dtypes.bfloat16)),
            )
        )
    return runs, chunk_tiles, T, per_core


def _build_program(n_nodes, NS, runs, chunk_tiles, T, qn, skip=()):
    """Emit the SPMD Bass program for one core (identical across cores)."""
    CH = len(chunk_tiles)
    last_w = NS - (CH - 1) * P
    nq = _cdiv(n_nodes, qn)

    nc = bacc.Bacc(
        "TRN2",
        target_bir_lowering=False,
        debug=False,
        num_devices=N_CORES,
        num_swdge_queues=4,
    )

    gd = 2 * D if "elem512" in skip else D
    xq = nc.dram_tensor("xq", [n_nodes, gd], bf16, kind="ExternalInput")
    idx16 = nc.dram_tensor("idx16", [P, T * 8], i16, kind="ExternalInput")
    dstr = nc.dram_tensor("dstr", [P, T], bf16, kind="ExternalInput")
    xrt = nc.dram_tensor("xrt", [P, NS], f32, kind="ExternalInput")
    w = nc.dram_tensor("w", [P, D], f32, kind="ExternalInput")
    bcol = nc.dram_tensor("b", [P, 1], f32, kind="ExternalInput")
    iota = nc.dram_tensor("iota", [P, P], bf16, kind="ExternalInput")
    outT = nc.dram_tensor("outT", [P, NS], f32, kind="ExternalOutput")

    # gather tile -> (run index, offset within run)
    tile_run = {}
    for ri, (g, q, ts, nt) in enumerate(runs):
        for j in range(nt):
            tile_run[ts + j] = (ri, j)
    # chunk -> first run index of its group's runs (for prefetch ordering)
    nruns = len(runs)

    with tile.TileContext(nc) as tc, ExitStack() as ctx:
        const = ctx.enter_context(tc.tile_pool(name="const", bufs=1))
        tbl = ctx.enter_context(tc.tile_pool(name="tbl", bufs=1))
        gbufs_n = 16
        if "bufs4" in skip:
            gbufs_n = 4
        if "bufs8" in skip:
            gbufs_n = 8
        gpool = ctx.enter_context(tc.tile_pool(name="gather", bufs=gbufs_n))
        opool = ctx.enter_context(tc.tile_pool(name="onehot", bufs=4))
        apsum = ctx.enter_context(tc.tile_pool(name="apsum", bufs=2, space="PSUM"))
        opsum = ctx.enter_context(tc.tile_pool(name="opsum", bufs=2, space="PSUM"))
        asb = ctx.enter_context(tc.tile_pool(name="asb", bufs=2))
        osb = ctx.enter_context(tc.tile_pool(name="osb", bufs=3))
        xrp = ctx.enter_context(tc.tile_pool(name="xrp", bufs=3))

        w_sb = const.tile([P, D], f32)
        nc.sync.dma_start(out=w_sb[:], in_=w[:])
        b_sb = const.tile([P, 1], f32)
        nc.sync.dma_start(out=b_sb[:], in_=bcol[:])
        io_sb = const.tile([P, P], bf16)
        nc.sync.dma_start(out=io_sb[:], in_=iota[:])
        idx_sb = tbl.tile([P, T * 8], i16)
        nc.sync.dma_start(out=idx_sb[:], in_=idx16[:])
        dstr_sb = tbl.tile([P, T], bf16)
        nc.sync.dma_start(out=dstr_sb[:], in_=dstr[:])

        gbufs = {}
        qctr = [0]
        if "gather" in skip:
            dummy = const.tile([P, 1, D], bf16, tag="dummy")
            nc.vector.memset(dummy[:], 0.0)
        if "onehot" in skip:
            ohc = const.tile([P, P], bf16, tag="ohc")
            nc.vector.memset(ohc[:], 0.0)

        def gather_run(ri):
            g, q, ts, nt = runs[ri]
            if "gather" in skip:
                gbufs[ri] = dummy
                return
            gb = gpool.tile([P, nt, gd], bf16, tag="gb")
            nq_queues = 2 if "queue2" in skip else 4
            for a in range(0, nt, MAXT):
                bnd = min(a + MAXT, nt)
                nc.gpsimd.dma_gather(
                    gb[:, a:bnd, :],
                    xq[q * qn : (q + 1) * qn, :],
                    idx_sb[:, (ts + a) * 8 : (ts + bnd) * 8],
                    (bnd - a) * P,
                    (bnd - a) * P,
                    gd,
                    elem_step=gd,
                    # round-robin sub-instructions across queues: smooths
                    # per-queue ring occupancy so desc-gen never stalls on
                    # await_space behind its own queue's draining burst
                    queue_num=qctr[0] % nq_queues,
                )
                qctr[0] += 1
            gbufs[ri] = gb

        obufs = {}

        def build_onehot(ob):
            """Batched one-hot build for consumption positions [ob*OG, ...)."""
            p0 = ob * OG
            kt = min(OG, T - p0)
            oh = opool.tile([P, OG, P], bf16, tag="oh")
            if "og1" in skip:
                for j in range(kt):
                    nc.vector.tensor_tensor(
                        out=oh[:, j, :],
                        in0=dstr_sb[:, p0 + j : p0 + j + 1].to_broadcast([P, P]),
                        in1=io_sb[:],
                        op=mybir.AluOpType.is_equal,
                    )
            else:
                nc.vector.tensor_tensor(
                    out=oh[:, 0:kt, :],
                    in0=dstr_sb[:, p0 : p0 + kt].unsqueeze(2).broadcast_to([P, kt, P]),
                    in1=io_sb[:].unsqueeze(1).broadcast_to([P, kt, P]),
                    op=mybir.AluOpType.is_equal,
                )
            obufs[ob] = oh

        cons_pos = 0
        for k in range(CH):
            tl = chunk_tiles[k]
            psum = apsum.tile([P, P], f32)
            if "mm" in skip:
                nc.vector.memset(psum[:], 0.0)
            for j, t in enumerate(tl):
                ri, jr = tile_run[t]
                if ri not in gbufs:
                    # issue this run and prefetch the next one
                    gather_run(ri)
                    if ri + 1 < nruns and ri + 1 not in gbufs:
                        gather_run(ri + 1)
                gb = gbufs[ri]
                ob = cons_pos // OG
                if "onehot" in skip:
                    oh_ap = ohc[:]
                else:
                    if ob not in obufs:
                        build_onehot(ob)
                    oh_ap = obufs[ob][:, cons_pos - ob * OG, :]
                if "mm" not in skip:
                    jg = 0 if "gather" in skip else jr
                    nc.tensor.matmul(
                        out=psum[:],
                        lhsT=gb[:, jg, 0:D],
                        rhs=oh_ap,
                        start=(j == 0),
                        stop=(j == len(tl) - 1),
                    )
                cons_pos += 1

            # aggT[f, d] (PSUM) -> SBUF on the Act engine, then
            # out2T[g, d] = sum_f W[f,g] * aggT[f,d]
            aggT = asb.tile([P, P], f32)
            nc.scalar.activation(
                out=aggT[:], in_=psum[:], func=mybir.ActivationFunctionType.Copy
            )
            if "notail" in skip:
                continue
            po = opsum.tile([P, P], f32)
            nc.tensor.matmul(out=po[:], lhsT=w_sb[:], rhs=aggT[:], start=True, stop=True)
            ot = osb.tile([P, P], f32)
            nc.scalar.activation(
                out=ot[:],
                in_=po[:],
                func=mybir.ActivationFunctionType.Relu,
                bias=b_sb[:],
            )
            wk = P if k < CH - 1 else last_w
            if "noresid" not in skip:
                xr = xrp.tile([P, P], f32)
                nc.sync.dma_start(out=xr[:, :wk], in_=xrt[:, k * P : k * P + wk])
                nc.vector.tensor_add(out=ot[:, :wk], in0=ot[:, :wk], in1=xr[:, :wk])
            nc.sync.dma_start(out=outT[:, k * P : k * P + wk], in_=ot[:, :wk])

    nc.compile()
    return nc


def _make_in_maps(x, edge_src, edge_dst, W, b, n_nodes, n_cores, qn, skip=()):
    NS = n_nodes // n_cores
    xf = np.ascontiguousarray(np.asarray(x, dtype=np.float32))
    x_hi = xf.astype(ml_dtypes.bfloat16)
    if "elem512" in skip:
        x_lo = (xf - x_hi.astype(np.float32)).astype(ml_dtypes.bfloat16)
        xq = np.ascontiguousarray(np.concatenate([x_hi, x_lo], axis=1))
    else:
        xq = np.ascontiguousarray(x_hi)

    runs, chunk_tiles, T, per_core = _build_tables(
        np.asarray(edge_src), np.asarray(edge_dst), n_nodes, n_cores, qn
    )

    w_np = np.ascontiguousarray(np.asarray(W, dtype=np.float32))
    b_np = np.ascontiguousarray(np.asarray(b, dtype=np.float32).reshape(P, 1))
    iota_np = np.ascontiguousarray(
        np.broadcast_to(np.arange(P), (P, P)).astype(ml_dtypes.bfloat16)
    )

    in_maps = []
    for c in range(n_cores):
        idx16_tbl, dstr_tbl = per_core[c]
        in_maps.append(
            {
                "xq": xq,
                "idx16": idx16_tbl,
                "dstr": dstr_tbl,
                "xrt": np.ascontiguousarray(xf[c * NS : (c + 1) * NS].T),
                "w": w_np,
                "b": b_np,
                "iota": iota_np,
            }
        )
    return in_maps, runs, chunk_tiles, T


def prepare(x, edge_src, edge_dst, W, b, skip=()):
    """Build (nc, in_maps) for the 8-core SPMD run."""
    n_nodes = int(np.asarray(x).shape[0])
    NS = n_nodes // N_CORES
    qn = min(QN, n_nodes)
    in_maps, runs, chunk_tiles, T = _make_in_maps(
        x, edge_src, edge_dst, W, b, n_nodes, N_CORES, qn, skip=skip
    )
    nc = _build_program(n_nodes, NS, runs, chunk_tiles, T, qn, skip=skip)
    return nc, in_maps


def kernel(x, edge_src, edge_dst, W, b):
    global last_results
    n_nodes = int(np.asarray(x).shape[0])
    NS = n_nodes // N_CORES

    nc, in_maps = prepare(x, edge_src, edge_dst, W, b)
    res = run_bass_kernel_spmd(nc, in_maps, core_ids=list(range(N_CORES)))
    last_results = res

    out = np.empty((n_nodes, D), np.float32)
    for c in range(N_CORES):
        out[c * NS : (c + 1) * NS] = res.results[c]["outT"].T
    return out
